# revision 35
# baseline (speedup 1.0000x reference)
"""Trainium2 Bass kernel for nn_MDRMWithCPRecon.

Sharding: pure data parallel over batch B=8 -> one batch element per
NeuronCore (8 cores). All parameters replicated. Each core computes the
full per-batch pipeline:

  x = cat(frm, oth)                 [512, 64, 64]
  Fm = lrelu(conv3x3(x, W3) + b3)   [256, 64, 64]   <- bulk of FLOPs
  U1/U2/U3 rank-4 softmax factors from pooled stats (tiny ops)
  spatial  = sigmoid(ws * U3 @ U2^T + bs)
  spectral = sigmoid(sigmoid(Wsa@mean + Wsm@max + biases))
  Wt = spectral x spatial
  fused    = a*Wt*frm + (1-a)*(1-Wt)*oth
  cp_recon = (Wr @ cp + br) * Wt + Fm,  cp = rank-4 CP(U1,U2,U3,lam)

Perf structure (vs the 218us f32r / 200us bf16 versions):
  - conv3x3 in bf16 (inputs + weights converted on host): 1 cyc/row
    matmul rate with fast-weight-load; input DMA bytes halved vs f32.
  - conv as 9-tap PSUM-accumulated matmuls, weight-major: each weight
    load feeds 4 consecutive matmuls into 4 parallel PSUM accumulators.
  - h-pooled stats via pairwise TT trees (unit-stride bf16, 2x DVE)
    instead of strided-view reduces.
  - adapter+U_gen collapsed on host: u = (Wu@Wa)@[avg;max] + (Wu@ba+bu),
    so pooled COLUMNS feed tiny scalar_tensor_tensor ops directly; no
    row transposes / 384-col adapter matmuls.
  - spectral folded into MT5 columns and into Pd once (tensor_scalar
    runs 4x on DVE; scalar_tensor_tensor measured 1x on HW - avoid).
  - sig folded into G5 rows (G5s) so the whole cp_recon path is PE
    matmul accumulation (MT5s @ G5s + I @ Fm) + one ACT copy per tile;
    plain DMA stores (no DMA-accumulate: its software-DGE descriptor
    flood cost ~18us of post-compute drain in the 218us version).
  - G5 outer product built in 4 chunks; spatial/sig tiles pipelined
    against the spectral/MT chains.
  - outputs stored bf16 and widened to f32 on the host.
"""

import numpy as np

import concourse.bacc as bacc
import concourse.bass as bass
import concourse.tile as tile
from concourse import mybir, bass_utils

F32 = mybir.dt.float32
BF16 = mybir.dt.bfloat16
AF = mybir.ActivationFunctionType
ALU = mybir.AluOpType
AX = mybir.AxisListType

B, C, H, W, K = 8, 256, 64, 64, 4
HW = H * W
NCORES = 8


def build_program(alpha, ws, bs):
    from concourse.masks import make_identity

    nc = bacc.Bacc("TRN2", target_bir_lowering=False, debug=False,
                   num_devices=NCORES)

    # ---- DRAM I/O (per core) ----
    xin_d = nc.dram_tensor("xin", [128, 4, 66, 66], BF16,
                           kind="ExternalInput")
    w3t_d = nc.dram_tensor("w3t", [128, 4, 9, 256], BF16,
                           kind="ExternalInput")
    bb_d = nc.dram_tensor("bb", [128, 4], F32, kind="ExternalInput")
    wkb_d = nc.dram_tensor("wkb", [128, 3, 3, 4], F32, kind="ExternalInput")
    wrt_d = nc.dram_tensor("wrt", [128, 2, 2, 128], BF16,
                           kind="ExternalInput")
    br_d = nc.dram_tensor("br", [1, 256], BF16, kind="ExternalInput")
    wsc_d = nc.dram_tensor("wsc", [128, 4, 2, 128], BF16,
                           kind="ExternalInput")
    lam_d = nc.dram_tensor("lam", [4, 1], F32, kind="ExternalInput")
    onesd = nc.dram_tensor("onesd", [1, HW], BF16, kind="ExternalInput")
    fused_o = nc.dram_tensor("fused", [2, 128, H, W], BF16,
                             kind="ExternalOutput")
    cpr_o = nc.dram_tensor("cpr", [2, 128, H, W], BF16, kind="ExternalOutput")

    with tile.TileContext(nc) as tc:
        _build_tile(tc, nc, make_identity, locals(), alpha, ws, bs)
    nc.compile()
    return nc


def _build_tile(tc, nc, make_identity, T, alpha, ws, bs):
    xin_d, w3t_d = T["xin_d"], T["w3t_d"]
    bb_d, wkb_d = T["bb_d"], T["wkb_d"]
    wrt_d, br_d, wsc_d, lam_d = T["wrt_d"], T["br_d"], T["wsc_d"], T["lam_d"]
    onesd = T["onesd"]
    fused_o, cpr_o = T["fused_o"], T["cpr_o"]

    import contextlib
    ctx = contextlib.ExitStack()
    consts = ctx.enter_context(tc.tile_pool(name="consts", bufs=1))
    stage = ctx.enter_context(tc.tile_pool(name="stage", bufs=2))
    ew = stage

    # conv weights + host-padded image in bf16 (contiguous DMAs)
    w3t_r = consts.tile([128, 4, 9, 256], BF16)
    xr = consts.tile([128, 4, 66, 66], BF16)

    # ---- input DMAs first: small kt0 gating pieces, then the rest ----
    nc.sync.dma_start(w3t_r[:, 0, 0:3], w3t_d[:, 0, 0:3])
    nc.sync.dma_start(xr[:, 0, 0:12, :], xin_d[:, 0, 0:12, :])
    nc.sync.dma_start(xr[:, 0, 12:23, :], xin_d[:, 0, 12:23, :])
    nc.sync.dma_start(w3t_r[:, 0, 3:9], w3t_d[:, 0, 3:9])
    nc.sync.dma_start(xr[:, 0, 23:34, :], xin_d[:, 0, 23:34, :])
    for kt in range(1, 4):
        nc.sync.dma_start(w3t_r[:, kt], w3t_d[:, kt])
        nc.sync.dma_start(xr[:, kt, 0:34, :], xin_d[:, kt, 0:34, :])
    for kt in range(4):
        nc.sync.dma_start(xr[:, kt, 34:66, :], xin_d[:, kt, 34:66, :])

    # ================= consts / layout =================
    ident = consts.tile([128, 128], F32)
    make_identity(nc, ident[:])
    identb = consts.tile([128, 128], BF16)
    nc.vector.tensor_copy(identb[:], ident[:])
    ones128 = consts.tile([128, 1], F32)
    nc.gpsimd.memset(ones128[:], 1.0)
    ones128b = consts.tile([128, 1], BF16)
    nc.gpsimd.memset(ones128b[:], 1.0)
    ones8 = consts.tile([1, 8], BF16)
    nc.gpsimd.memset(ones8[:], 1.0)
    ones4b = consts.tile([4, 128], BF16)
    nc.gpsimd.memset(ones4b[:], 1.0)

    # ---- small weights ----
    bb_sb = consts.tile([128, 4], F32)
    nc.gpsimd.dma_start(bb_sb[:], bb_d[:])
    lam_sb = consts.tile([4, 1], F32)
    nc.gpsimd.dma_start(lam_sb[:], lam_d[:])
    wkb = consts.tile([128, 3, 3, 4], F32)
    nc.gpsimd.dma_start(wkb[:], wkb_d[:])
    wrt_b = consts.tile([128, 2, 2, 128], BF16)
    nc.gpsimd.dma_start(wrt_b[:], wrt_d[:])
    wsc_b = consts.tile([128, 4, 2, 128], BF16)
    nc.gpsimd.dma_start(wsc_b[:], wsc_d[:])
    b3_sb = bb_sb[:, 0:2]                          # f32 conv bias
    bsc_sb = bb_sb[:, 2:4]                         # f32 spectral bias

    # ---- persistent intermediates ----
    Fm = consts.tile([128, 2, HW], BF16)          # conv output, (ct, h*64+w)
    Pd = consts.tile([128, 2, HW], BF16)          # alpha*frm - (1-alpha)*oth
    PdS = consts.tile([128, 2, HW], BF16)         # Pd * spectral (tail)
    G5s = consts.tile([5, HW], BF16)              # G5 * sig (tail)
    t2 = consts.tile([128, 2, HW], BF16)          # (1-alpha)*oth
    sums1 = consts.tile([128, 2, 2, 4], F32)      # ACT accum per 512-tile
    S3s = consts.tile([128, 2, 64], BF16)         # mode3 h-sums per ct
    S3m = consts.tile([128, 2, 64], BF16)         # mode3 h-maxes per ct
    # mode2 partials, w-major w/ slot innermost: 0,1=ct0 2=ct1h0 3:7=ct1h1
    pp_s = consts.tile([128, 64, 7], BF16)
    pp_m = consts.tile([128, 64, 7], BF16)
    stack4 = consts.tile([128, 4], F32)           # [sum1 ct0/1 | max1 ct0/1]
    Um1 = consts.tile([128, 2, 4], BF16)          # softmaxed U1 (ct, k)
    Um23 = consts.tile([64, 2, 4], BF16)          # softmaxed U2/U3 (m, k)
    U1T = consts.tile([4, 2, 128], BF16)
    UWH = consts.tile([4, 128], BF16)             # 0:64=U2T(w) 64:128=U3T(h)
    G5 = consts.tile([5, HW], BF16)               # G[r,hw]; row4 = ones
    nc.sync.dma_start(G5[4:5, :], onesd[:])
    MT5 = consts.tile([5, 256], BF16)             # (Wr U1 lam)^T; row4 = br
    nc.sync.dma_start(MT5[4:5, :], br_d[:])
    MT5s = consts.tile([5, 256], BF16)            # MT5 * spectral (folded)
    gag = consts.tile([128, 4], BF16)             # [ga ct0/1 | gm ct0/1]
    spectral = consts.tile([128, 2], F32)
    mxr2 = consts.tile([64, 1], F32)
    mxr3 = consts.tile([64, 1], F32)
    utm1 = consts.tile([128, 2, 4], F32)
    utm23 = consts.tile([64, 2, 4], F32)

    # ---- blend precompute on DVE (runs during early conv) ----
    for c2 in range(2):
        nc.vector.tensor_scalar(
            t2[:, c2].rearrange("p (h w) -> p h w", h=64),
            xr[:, 2 + c2, 1:65, 1:65], float(1.0 - alpha), None,
            op0=ALU.mult)
        nc.vector.scalar_tensor_tensor(
            Pd[:, c2].rearrange("p (h w) -> p h w", h=64),
            xr[:, c2, 1:65, 1:65], float(alpha),
            t2[:, c2].rearrange("p (h w) -> p h w", h=64),
            op0=ALU.mult, op1=ALU.subtract)

    def htree(blk, nh, slot):
        """Pairwise-add/max tree over the h axis of blk [128, nh, 64],
        writing pp_s/pp_m[:, :, slot]."""
        for op, pp in ((ALU.add, pp_s), (ALU.max, pp_m)):
            nm = "s" if op == ALU.add else "m"
            cur = blk
            n = nh
            while n > 2:
                nxt = ew.tile([128, n // 2, 64], BF16, tag=f"ht{nm}{n}")
                nc.vector.tensor_tensor(nxt[:], cur[:, 0:n // 2],
                                        cur[:, n // 2:n], op=op)
                cur = nxt
                n //= 2
            nc.vector.tensor_tensor(pp[:, :, slot], cur[:, 0], cur[:, 1],
                                    op=op)

    # batched softmax over k; exp via 4th-order Taylor (|u| ~ 0.1)
    def softmax4(ut, Uo, p, tagp, w=2):
        h1 = ew.tile([p, w, 4], F32, tag=f"h1{tagp}")
        h2 = ew.tile([p, w, 4], F32, tag=f"h2{tagp}")
        nc.vector.tensor_scalar(h1[:], ut[:], 0.25, 1.0, op0=ALU.mult,
                                op1=ALU.add)
        nc.vector.tensor_tensor(h2[:], h1[:], ut[:], op=ALU.mult)
        nc.vector.tensor_scalar(h1[:], h2[:], 1.0 / 3.0, 1.0,
                                op0=ALU.mult, op1=ALU.add)
        nc.vector.tensor_tensor(h2[:], h1[:], ut[:], op=ALU.mult)
        nc.vector.tensor_scalar(h1[:], h2[:], 0.5, 1.0, op0=ALU.mult,
                                op1=ALU.add)
        nc.vector.tensor_tensor(h2[:], h1[:], ut[:], op=ALU.mult)
        nc.vector.tensor_scalar(h1[:], h2[:], 1.0, 1.0, op0=ALU.mult,
                                op1=ALU.add)
        ssum = ew.tile([p, w], F32, tag=f"ss{tagp}")
        nc.vector.tensor_reduce(ssum[:], h1[:], axis=AX.X, op=ALU.add)
        rcp = ew.tile([p, w], F32, tag=f"rc{tagp}")
        nc.vector.reciprocal(rcp[:], ssum[:])
        nc.vector.tensor_tensor(Uo[:], h1[:],
                                rcp[:, :, None].broadcast_to([p, w, 4]),
                                op=ALU.mult)


    # ================= conv3x3 (PE) + streaming stats =================
    CHUNKS = [(0, 0), (1, 0), (0, 1), (1, 1)]     # (ct, half); h0 halves first
    with tc.tile_pool(name="ps_conv", bufs=8, space="PSUM") as ps_conv:
        for ci, (ct, hf) in enumerate(CHUNKS):
            if ci < 3:
                pss = [ps_conv.tile([128, 512], F32, tag="conv",
                                    name=f"cv{ci}{p}") for p in range(4)]
                idx = 0
                for kt in range(4):
                    for t in range(9):
                        dy, dx = t // 3, t % 3
                        for pq in range(4):
                            r0 = hf * 32 + pq * 8 + dy
                            nc.tensor.matmul(
                                pss[pq][:],
                                w3t_r[:, kt, t, ct * 128:(ct + 1) * 128],
                                xr[:, kt, r0: r0 + 8, dx: dx + 64],
                                start=(idx == 0), stop=(idx == 35))
                        idx += 1
                for pq in range(4):
                    o0 = hf * 2048 + pq * 512
                    nc.scalar.activation(Fm[:, ct, o0:o0 + 512], pss[pq][:],
                                         AF.Prelu, bias=b3_sb[:, ct:ct + 1],
                                         alpha=0.01,
                                         accum_out=sums1[:, ct, hf, pq:pq + 1])
                fm_sl = Fm[:, ct, hf * 2048:(hf + 1) * 2048]
                blk = fm_sl.rearrange("p (h w) -> p h w", h=32)
                with nc.allow_low_precision(reason="bf16 pooled partials; "
                                            "0.4% fine for attention gates"):
                    nc.vector.tensor_reduce(S3s[:, ct, hf * 32:(hf + 1) * 32],
                                            blk, axis=AX.X, op=ALU.add)
                nc.vector.tensor_reduce(S3m[:, ct, hf * 32:(hf + 1) * 32],
                                        blk, axis=AX.X, op=ALU.max)
                htree(blk, 32, ct * 2 + hf)
            else:
                for pq in range(4):
                    ps = ps_conv.tile([128, 512], F32, tag="conv",
                                      name=f"cv3{pq}")
                    idx = 0
                    for kt in range(4):
                        for t in range(9):
                            dy, dx = t // 3, t % 3
                            r0 = hf * 32 + pq * 8 + dy
                            nc.tensor.matmul(
                                ps[:],
                                w3t_r[:, kt, t, ct * 128:(ct + 1) * 128],
                                xr[:, kt, r0: r0 + 8, dx: dx + 64],
                                start=(idx == 0), stop=(idx == 35))
                            idx += 1
                    o0 = hf * 2048 + pq * 512
                    nc.scalar.activation(Fm[:, ct, o0:o0 + 512], ps[:],
                                         AF.Prelu, bias=b3_sb[:, ct:ct + 1],
                                         alpha=0.01,
                                         accum_out=sums1[:, ct, hf, pq:pq + 1])
                    sl = Fm[:, ct, o0:o0 + 512]
                    blk = sl.rearrange("p (h w) -> p h w", h=8)
                    s0 = hf * 32 + pq * 8
                    with nc.allow_low_precision(reason="bf16 pooled "
                                                "partials; 0.4% fine"):
                        nc.vector.tensor_reduce(S3s[:, ct, s0:s0 + 8], blk,
                                                axis=AX.X, op=ALU.add)
                    nc.vector.tensor_reduce(S3m[:, ct, s0:s0 + 8], blk,
                                            axis=AX.X, op=ALU.max)
                    htree(blk, 8, 3 + pq)

    # ================= pooled-stat columns -> U factors =================
    sigp = ctx.enter_context(tc.tile_pool(name="sigp", bufs=3))
    tailp = ctx.enter_context(tc.tile_pool(name="tailp", bufs=3))
    ps_sp = ctx.enter_context(tc.tile_pool(name="ps_sp", bufs=2,
                                           space="PSUM"))
    sps = []

    def spat_q(q):
        sp_ps = ps_sp.tile([128, 1024], F32, tag="sp", name=f"sp{q}")
        for hx in range(2):
            h0 = q * 1024 + hx * 512
            nc.tensor.matmul(sp_ps[:, hx * 512:(hx + 1) * 512],
                             ones4b[:], G5[0:4, h0:h0 + 512],
                             start=True, stop=True)
        sps.append(sp_ps)

    sigs = []

    def sig_q(q):
        sig = sigp.tile([128, 1024], BF16, tag="sig", name=f"sig{q}")
        nc.scalar.activation(sig[:], sps[q][:], AF.Sigmoid,
                             scale=float(ws), bias=float(bs))
        sigs.append(sig)

    def g5s_q(q):
        qs = slice(q * 1024, (q + 1) * 1024)
        nc.vector.tensor_tensor(G5s[:, qs], G5[:, qs], sigs[q][0:5, :],
                                op=ALU.mult)

    with tc.tile_pool(name="ps_mid", bufs=1, space="PSUM") as ps_mid:
        sm = ps_mid.tile([128, 512], F32, tag="sm")
        tpz = ps_mid.tile([64, 6, 128], BF16, tag="tpz")
        t3 = tpz[0:4, 0:3, :]
        mt2 = ps_mid.tile([4, 2, 128], F32, tag="mt2")
        tp = tpz[:, 3:5, :]
        avgc = sm[0:64, 26:28]
        spz = ps_mid.tile([5, 2, 256], F32, tag="spz")
        sprow = spz[0:1, 0, :]
        spec5 = spz[:, 1, :]

        # preload the sigmoid ACT table while the ACT queue is idle
        warm1 = ew.tile([1, 1], F32, tag="warm1")
        nc.scalar.activation(warm1[:], ones128[0:1, 0:1], AF.Sigmoid)

        # mode1 columns: sums + maxes per ct
        nc.vector.tensor_reduce(stack4[:, 0:2],
                                sums1[:].rearrange("p a b c -> p a (b c)"),
                                axis=AX.X, op=ALU.add)
        nc.vector.tensor_reduce(stack4[:, 2:4], S3m[:], axis=AX.X, op=ALU.max)
        # mode2 max column: combine slots, transpose, reduce
        pmx = ew.tile([128, 64], BF16, tag="pmx")
        nc.vector.tensor_reduce(pmx[:], pp_m[:], axis=AX.X, op=ALU.max)
        nc.tensor.transpose(tp[:, 0, :], pmx[:], identb[:])
        nc.vector.tensor_reduce(mxr2[:], tp[:, 0, :], axis=AX.X, op=ALU.max)
        # mode3 max column
        qmx = ew.tile([128, 64], BF16, tag="qmx")
        nc.vector.tensor_tensor(qmx[:], S3m[:, 0, :], S3m[:, 1, :],
                                op=ALU.max)
        nc.tensor.transpose(tp[:, 1, :], qmx[:], identb[:])
        nc.vector.tensor_reduce(mxr3[:], tp[:, 1, :], axis=AX.X, op=ALU.max)
        # mode2/3 avg columns via cross-partition matmuls
        for s in range(7):
            nc.tensor.matmul(avgc[:, 0:1], pp_s[:, :, s], ones128b[:],
                             start=(s == 0), stop=(s == 6))
        for cti in range(2):
            nc.tensor.matmul(avgc[:, 1:2], S3s[:, cti, :], ones128b[:],
                             start=(cti == 0), stop=(cti == 1))

        # u columns: u = wk0*avg + wk1*max + bk  (adapters folded on host)
        hlf = ew.tile([128, 2, 4], F32, tag="hlf")
        for cti in range(2):
            nc.vector.scalar_tensor_tensor(
                hlf[:, cti, :], wkb[:, 0, 1, :], stack4[:, 2 + cti:3 + cti],
                wkb[:, 0, 2, :], op0=ALU.mult, op1=ALU.add)
            nc.vector.scalar_tensor_tensor(
                utm1[:, cti, :], wkb[:, 0, 0, :], stack4[:, cti:cti + 1],
                hlf[:, cti, :], op0=ALU.mult, op1=ALU.add)
        hlf2 = ew.tile([64, 2, 4], F32, tag="hlf2")
        for mi, mxc in ((0, mxr2), (1, mxr3)):
            nc.vector.scalar_tensor_tensor(
                hlf2[:, mi, :], wkb[0:64, 1 + mi, 1, :], mxc[:],
                wkb[0:64, 1 + mi, 2, :], op0=ALU.mult, op1=ALU.add)
            nc.vector.scalar_tensor_tensor(
                utm23[:, mi, :], wkb[0:64, 1 + mi, 0, :],
                avgc[:, mi:mi + 1], hlf2[:, mi, :],
                op0=ALU.mult, op1=ALU.add)

        softmax4(utm1, Um1, 128, "a")
        softmax4(utm23, Um23, 64, "b")

        # transpose U columns to rank-rows
        for cti in range(2):
            nc.tensor.transpose(t3[:, cti, :], Um1[:, cti, :], identb[:])
        nc.tensor.transpose(t3[:, 2, 0:64], Um23[:, 0, :],
                            identb[0:64, 0:64])
        nc.tensor.transpose(t3[:, 2, 64:128], Um23[:, 1, :],
                            identb[0:64, 0:64])
        nc.scalar.copy(U1T[:], t3[:, 0:2, :])
        nc.scalar.copy(UWH[:], t3[:, 2, :])

        # spectral input stats: F_spe = U1 @ [U2;U3]^T, then mean/max
        for cti in range(2):
            fps = sm[:, 128 + cti * 128:256 + cti * 128]
            nc.tensor.matmul(fps, U1T[:, cti, :], UWH[:], start=True,
                             stop=True)
            with nc.allow_low_precision(reason="128-val mean into bf16 for "
                                        "double-sigmoid gate; 0.4% is fine"):
                nc.vector.tensor_reduce(gag[:, cti:cti + 1], fps, axis=AX.X,
                                        op=ALU.add)
            nc.vector.tensor_reduce(gag[:, 2 + cti:3 + cti], fps, axis=AX.X,
                                    op=ALU.max)

        # G5 construction + first spatial/sig tiles: only need UWH;
        # overlap the spectral/MT chains below on other engines
        def g5_chunk(q):
            qs = slice(q * 1024, (q + 1) * 1024)
            nc.vector.tensor_tensor(
                G5[0:4, qs].rearrange("p (h w) -> p h w", h=16),
                UWH[:, 64 + q * 16:64 + (q + 1) * 16][:, :, None]
                .broadcast_to([4, 16, 64]),
                UWH[:, 0:64][:, None, :].broadcast_to([4, 16, 64]),
                op=ALU.mult)

        g5_chunk(0)
        g5_chunk(1)
        g5_chunk(2)
        g5_chunk(3)
        spat_q(0)
        spat_q(1)
        sig_q(0)
        sig_q(1)
        g5s_q(0)
        g5s_q(1)

        # spectral attention (double sigmoid)
        for mm in range(2):
            sp_ps = sm[:, 24 + mm:25 + mm]
            for kk in range(4):
                nc.tensor.matmul(sp_ps, wsc_b[:, kk, mm, :],
                                 gag[:, kk:kk + 1], start=(kk == 0),
                                 stop=(kk == 3))
            stmp = ew.tile([128, 1], F32, tag=f"stmp{mm}")
            nc.scalar.activation(stmp[:], sp_ps, AF.Sigmoid,
                                 bias=bsc_sb[:, mm:mm + 1])
            nc.scalar.activation(spectral[:, mm:mm + 1], stmp[:], AF.Sigmoid)


        # MT = (Wr @ U1 diag(lam))^T, then fold spectral into columns
        for mm in range(2):
            m_ps = sm[:, 16 + mm * 4:20 + mm * 4]
            for kk in range(2):
                nc.tensor.matmul(m_ps, wrt_b[:, kk, mm, :], Um1[:, kk, :],
                                 start=(kk == 0), stop=(kk == 1))
            m_sb = ew.tile([128, 4], F32, tag=f"msb{mm}")
            nc.scalar.copy(m_sb[:], m_ps)
            nc.tensor.transpose(mt2[:, mm, :], m_sb[:], ident[:])
            with nc.allow_low_precision(reason="bf16 CP factors; gates "
                                        "tolerate 0.4%"):
                nc.vector.tensor_scalar(MT5[0:4, mm * 128:(mm + 1) * 128],
                                        mt2[:, mm, :], lam_sb[:], None,
                                        op0=ALU.mult)
        nc.tensor.transpose(sprow[0:1, 0:128], spectral[:, 0:1], ident[:])
        nc.tensor.transpose(sprow[0:1, 128:256], spectral[:, 1:2], ident[:])
        sprow_sb = ew.tile([1, 256], BF16, tag="sprow_sb")
        nc.scalar.copy(sprow_sb[:], sprow[:])
        nc.tensor.matmul(spec5[:], ones8[0:1, 0:5], sprow_sb[:],
                         start=True, stop=True)
        with nc.allow_low_precision(reason="bf16 recon factors; 0.4% fine"):
            nc.vector.tensor_tensor(MT5s[:], MT5[:], spec5[:], op=ALU.mult)

    # ================= final elementwise stage =================
    # fused = (sig*spec)*Pd + t2 ; cp_recon = (MT5s G)*sig + Fm
    with tc.tile_pool(name="ps_tail", bufs=2, space="PSUM") as ps_tail:
        def pds_prep():
            # PdS = Pd * spectral (per-partition scale; tensor_scalar 4x)
            for ct in range(2):
                nc.vector.tensor_scalar(PdS[:, ct, :], Pd[:, ct, :],
                                        spectral[:, ct:ct + 1], None,
                                        op0=ALU.mult)

        def tail_q(q):
            sig = sigs[q]
            sigv = sig[:].rearrange("p (h w) -> p h w", h=16)
            qs = slice(q * 1024, (q + 1) * 1024)
            for ct in range(2):
                # rc = (MT5s @ G5s) + Fm  entirely in PSUM (PE-only cpr)
                rc = ps_tail.tile([128, 1024], F32, tag="rc",
                                  name=f"rc{q}{ct}")
                for hx in range(2):
                    h0 = q * 1024 + hx * 512
                    nc.tensor.matmul(rc[:, hx * 512:(hx + 1) * 512],
                                     MT5s[:, ct * 128:(ct + 1) * 128],
                                     G5s[:, h0:h0 + 512], start=True,
                                     stop=False)
                    nc.tensor.matmul(rc[:, hx * 512:(hx + 1) * 512],
                                     identb[:], Fm[:, ct, h0:h0 + 512],
                                     start=False, stop=True)
                rcb = sigp.tile([128, 1024], BF16, tag="rcb",
                                name=f"rcb{q}{ct}")
                nc.scalar.copy(rcb[:], rc[:])
                nc.sync.dma_start(cpr_o[ct, :, q * 16:(q + 1) * 16, :],
                                  rcb[:].rearrange("p (h w) -> p h w",
                                                   h=16))
                A2 = tailp.tile([128, 16, 64], BF16, tag="A2",
                                name=f"A2{q}{ct}")
                nc.vector.tensor_tensor(
                    A2[:], PdS[:, ct, qs].rearrange("p (h w) -> p h w", h=16),
                    sigv, op=ALU.mult)
                fu = tailp.tile([128, 16, 64], BF16, tag="fu",
                                name=f"fu{q}{ct}")
                nc.vector.tensor_tensor(
                    fu[:], A2[:],
                    t2[:, ct, qs].rearrange("p (h w) -> p h w", h=16),
                    op=ALU.add)
                nc.sync.dma_start(fused_o[ct, :, q * 16:(q + 1) * 16, :],
                                  fu[:])

        pds_prep()
        tail_q(0)
        spat_q(2)
        sig_q(2)
        g5s_q(2)
        tail_q(1)
        spat_q(3)
        sig_q(3)
        g5s_q(3)
        tail_q(2)
        tail_q(3)
    ctx.close()


def _prep_weights(W3, b3, Wa1, ba1, Wa2, ba2, Wa3, ba3, Wu, bu, Wr, br,
                  Wsa, bsa, Wsm, bsm):
    f = np.float32
    # w3t[p, kt, t, co] = W3[co, kt*128+p, dy, dx]
    w3t = np.ascontiguousarray(
        W3.reshape(C, 4, 128, 9).transpose(2, 1, 3, 0)).astype(f)
    b3h = np.ascontiguousarray(b3.reshape(2, 128).T).astype(f)
    # adapter + U_gen collapsed: u = (Wu@Wa)@[avg;max] + (Wu@ba + bu)
    wkb = np.zeros((3, 3, 4), f)
    for mi, (Wa, ba, div) in enumerate(
            ((Wa1, ba1, HW), (Wa2, ba2, C * H), (Wa3, ba3, C * W))):
        wk = Wu @ Wa                               # [4, 2]
        wkb[mi, 0] = wk[:, 0] / float(div)
        wkb[mi, 1] = wk[:, 1]
        wkb[mi, 2] = Wu @ ba + bu
    wkb = np.ascontiguousarray(
        np.broadcast_to(wkb[None], (128, 3, 3, 4))).astype(f)
    # wrt[p, kk, mm, m] = Wr[mm*128+m, kk*128+p]
    wrt = np.ascontiguousarray(
        Wr.reshape(2, 128, 2, 128).transpose(3, 2, 0, 1)).astype(f)
    brh = br.reshape(1, 256).astype(f)
    # wsc[p, kk, mm, m]: kk<2 -> Wsa/128 (mean folded), kk>=2 -> Wsm
    wsa_r = (Wsa / 128.0).reshape(2, 128, 2, 128).transpose(3, 2, 0, 1)
    wsm_r = Wsm.reshape(2, 128, 2, 128).transpose(3, 2, 0, 1)
    wsc = np.ascontiguousarray(
        np.concatenate([wsa_r, wsm_r], axis=1)).astype(f)
    bsc = np.ascontiguousarray((bsa + bsm).reshape(2, 128).T).astype(f)
    import ml_dtypes
    bf = ml_dtypes.bfloat16
    return dict(w3t=w3t.astype(bf), bb=np.ascontiguousarray(
                    np.concatenate([b3h, bsc], axis=1)),
                wkb=wkb, wrt=wrt.astype(bf), br=brh.astype(bf),
                wsc=wsc.astype(bf), onesd=np.ones((1, HW), bf))


_CACHE = {}


def kernel(frm_feat, other_feat, W3, b3, Wa1, ba1, Wa2, ba2, Wa3, ba3,
           Wu, bu, Wr, br, ws, bs, Wsa, bsa, Wsm, bsm, alpha, lam,
           _trace=False, _tmpdir=None):
    import ml_dtypes
    bf = ml_dtypes.bfloat16
    frm_feat = np.asarray(frm_feat, np.float32)
    other_feat = np.asarray(other_feat, np.float32)
    key = (float(alpha), float(ws), float(bs))
    if key not in _CACHE:
        _CACHE[key] = build_program(float(alpha), float(ws), float(bs))
    nc = _CACHE[key]

    wd = _prep_weights(np.asarray(W3), np.asarray(b3), np.asarray(Wa1),
                       np.asarray(ba1), np.asarray(Wa2), np.asarray(ba2),
                       np.asarray(Wa3), np.asarray(ba3), np.asarray(Wu),
                       np.asarray(bu), np.asarray(Wr), np.asarray(br),
                       np.asarray(Wsa), np.asarray(bsa), np.asarray(Wsm),
                       np.asarray(bsm))
    wd["lam"] = np.asarray(lam, np.float32).reshape(4, 1)

    in_maps = []
    for b_i in range(NCORES):
        m = dict(wd)
        xin = np.zeros((128, 4, 66, 66), bf)
        xin[:, 0:2, 1:65, 1:65] = frm_feat[b_i].reshape(
            2, 128, 64, 64).transpose(1, 0, 2, 3).astype(bf)
        xin[:, 2:4, 1:65, 1:65] = other_feat[b_i].reshape(
            2, 128, 64, 64).transpose(1, 0, 2, 3).astype(bf)
        m["xin"] = xin
        in_maps.append(m)

    res = bass_utils.run_bass_kernel_spmd(
        nc, in_maps, core_ids=list(range(NCORES)), trace=_trace,
        tmpdir=_tmpdir)
    fused = np.stack([
        np.asarray(res.results[i]["fused"]).astype(np.float32).reshape(C, H, W)
        for i in range(NCORES)])
    cpr = np.stack([
        np.asarray(res.results[i]["cpr"]).astype(np.float32).reshape(C, H, W)
        for i in range(NCORES)])
    kernel._last_exec_time_ns = res.exec_time_ns
    kernel._last_results = res
    return fused, cpr


# revision 36
# speedup vs baseline: 1.0109x; 1.0109x over previous
"""Trainium2 Bass kernel for nn_MDRMWithCPRecon.

Sharding: pure data parallel over batch B=8 -> one batch element per
NeuronCore (8 cores). All parameters replicated. Each core computes the
full per-batch pipeline:

  x = cat(frm, oth)                 [512, 64, 64]
  Fm = lrelu(conv3x3(x, W3) + b3)   [256, 64, 64]   <- bulk of FLOPs
  U1/U2/U3 rank-4 softmax factors from pooled stats (tiny ops)
  spatial  = sigmoid(ws * U3 @ U2^T + bs)
  spectral = sigmoid(sigmoid(Wsa@mean + Wsm@max + biases))
  Wt = spectral x spatial
  fused    = a*Wt*frm + (1-a)*(1-Wt)*oth
  cp_recon = (Wr @ cp + br) * Wt + Fm,  cp = rank-4 CP(U1,U2,U3,lam)

Perf structure (vs the 218us f32r / 200us bf16 versions):
  - conv3x3 in bf16 (inputs + weights converted on host): 1 cyc/row
    matmul rate with fast-weight-load; input DMA bytes halved vs f32.
  - conv as 9-tap PSUM-accumulated matmuls, weight-major: each weight
    load feeds 4 consecutive matmuls into 4 parallel PSUM accumulators.
  - h-pooled stats via pairwise TT trees (unit-stride bf16, 2x DVE)
    instead of strided-view reduces.
  - adapter+U_gen collapsed on host: u = (Wu@Wa)@[avg;max] + (Wu@ba+bu),
    so pooled COLUMNS feed tiny scalar_tensor_tensor ops directly; no
    row transposes / 384-col adapter matmuls.
  - spectral folded into MT5 columns and into Pd once (tensor_scalar
    runs 4x on DVE; scalar_tensor_tensor measured 1x on HW - avoid).
  - sig folded into G5 rows (G5s) so the whole cp_recon path is PE
    matmul accumulation (MT5s @ G5s + I @ Fm) + one ACT copy per tile;
    plain DMA stores (no DMA-accumulate: its software-DGE descriptor
    flood cost ~18us of post-compute drain in the 218us version).
  - G5 outer product built in 4 chunks; spatial/sig tiles pipelined
    against the spectral/MT chains.
  - outputs stored bf16 and widened to f32 on the host.
"""

import numpy as np

import concourse.bacc as bacc
import concourse.bass as bass
import concourse.tile as tile
from concourse import mybir, bass_utils

F32 = mybir.dt.float32
BF16 = mybir.dt.bfloat16
AF = mybir.ActivationFunctionType
ALU = mybir.AluOpType
AX = mybir.AxisListType

B, C, H, W, K = 8, 256, 64, 64, 4
HW = H * W
NCORES = 8


def build_program(alpha, ws, bs):
    from concourse.masks import make_identity

    nc = bacc.Bacc("TRN2", target_bir_lowering=False, debug=False,
                   num_devices=NCORES)

    # ---- DRAM I/O (per core) ----
    xin_d = nc.dram_tensor("xin", [128, 4, 66, 66], BF16,
                           kind="ExternalInput")
    w3t_d = nc.dram_tensor("w3t", [128, 4, 9, 256], BF16,
                           kind="ExternalInput")
    bb_d = nc.dram_tensor("bb", [128, 4], F32, kind="ExternalInput")
    wkb_d = nc.dram_tensor("wkb", [128, 3, 3, 4], F32, kind="ExternalInput")
    wrt_d = nc.dram_tensor("wrt", [128, 2, 2, 128], BF16,
                           kind="ExternalInput")
    br_d = nc.dram_tensor("br", [1, 256], BF16, kind="ExternalInput")
    wsc_d = nc.dram_tensor("wsc", [128, 4, 2, 128], BF16,
                           kind="ExternalInput")
    lam_d = nc.dram_tensor("lam", [4, 1], F32, kind="ExternalInput")
    onesd = nc.dram_tensor("onesd", [1, HW], BF16, kind="ExternalInput")
    fused_o = nc.dram_tensor("fused", [2, 128, H, W], BF16,
                             kind="ExternalOutput")
    cpr_o = nc.dram_tensor("cpr", [2, 128, H, W], BF16, kind="ExternalOutput")

    with tile.TileContext(nc) as tc:
        _build_tile(tc, nc, make_identity, locals(), alpha, ws, bs)
    nc.compile()
    return nc


def _build_tile(tc, nc, make_identity, T, alpha, ws, bs):
    xin_d, w3t_d = T["xin_d"], T["w3t_d"]
    bb_d, wkb_d = T["bb_d"], T["wkb_d"]
    wrt_d, br_d, wsc_d, lam_d = T["wrt_d"], T["br_d"], T["wsc_d"], T["lam_d"]
    onesd = T["onesd"]
    fused_o, cpr_o = T["fused_o"], T["cpr_o"]

    import contextlib
    ctx = contextlib.ExitStack()
    consts = ctx.enter_context(tc.tile_pool(name="consts", bufs=1))
    stage = ctx.enter_context(tc.tile_pool(name="stage", bufs=2))
    ew = stage

    # conv weights + host-padded image in bf16 (contiguous DMAs)
    w3t_r = consts.tile([128, 4, 9, 256], BF16)
    xr = consts.tile([128, 4, 66, 66], BF16)

    # ---- input DMAs first: small kt0 gating pieces, then the rest ----
    nc.sync.dma_start(w3t_r[:, 0, 0:3], w3t_d[:, 0, 0:3])
    nc.sync.dma_start(xr[:, 0, 0:12, :], xin_d[:, 0, 0:12, :])
    nc.sync.dma_start(xr[:, 0, 12:23, :], xin_d[:, 0, 12:23, :])
    nc.sync.dma_start(w3t_r[:, 0, 3:9], w3t_d[:, 0, 3:9])
    nc.sync.dma_start(xr[:, 0, 23:34, :], xin_d[:, 0, 23:34, :])
    for kt in range(1, 4):
        nc.sync.dma_start(w3t_r[:, kt], w3t_d[:, kt])
        nc.sync.dma_start(xr[:, kt, 0:34, :], xin_d[:, kt, 0:34, :])
    for kt in range(4):
        nc.sync.dma_start(xr[:, kt, 34:66, :], xin_d[:, kt, 34:66, :])

    # ================= consts / layout =================
    ident = consts.tile([128, 128], F32)
    make_identity(nc, ident[:])
    identb = consts.tile([128, 128], BF16)
    nc.vector.tensor_copy(identb[:], ident[:])
    ones128 = consts.tile([128, 1], F32)
    nc.gpsimd.memset(ones128[:], 1.0)
    ones128b = consts.tile([128, 1], BF16)
    nc.gpsimd.memset(ones128b[:], 1.0)
    ones8 = consts.tile([1, 8], BF16)
    nc.gpsimd.memset(ones8[:], 1.0)
    ones4b = consts.tile([4, 128], BF16)
    nc.gpsimd.memset(ones4b[:], 1.0)

    # ---- small weights ----
    bb_sb = consts.tile([128, 4], F32)
    nc.gpsimd.dma_start(bb_sb[:], bb_d[:])
    lam_sb = consts.tile([4, 1], F32)
    nc.gpsimd.dma_start(lam_sb[:], lam_d[:])
    wkb = consts.tile([128, 3, 3, 4], F32)
    nc.gpsimd.dma_start(wkb[:], wkb_d[:])
    wrt_b = consts.tile([128, 2, 2, 128], BF16)
    nc.gpsimd.dma_start(wrt_b[:], wrt_d[:])
    wsc_b = consts.tile([128, 4, 2, 128], BF16)
    nc.gpsimd.dma_start(wsc_b[:], wsc_d[:])
    b3_sb = bb_sb[:, 0:2]                          # f32 conv bias
    bsc_sb = bb_sb[:, 2:4]                         # f32 spectral bias

    # ---- persistent intermediates ----
    Fm = consts.tile([128, 2, HW], BF16)          # conv output, (ct, h*64+w)
    Pd = consts.tile([128, 2, HW], BF16)          # alpha*frm - (1-alpha)*oth
    PdS = consts.tile([128, 2, HW], BF16)         # Pd * spectral (tail)
    G5s = consts.tile([5, HW], BF16)              # G5 * sig (tail)
    t2 = consts.tile([128, 2, HW], BF16)          # (1-alpha)*oth
    sums1 = consts.tile([128, 2, 2, 4], F32)      # ACT accum per 512-tile
    S3s = consts.tile([128, 2, 64], BF16)         # mode3 h-sums per ct
    S3m = consts.tile([128, 2, 64], BF16)         # mode3 h-maxes per ct
    # mode2 partials, w-major w/ slot innermost: 0,1=ct0 2=ct1h0 3:7=ct1h1
    pp_s = consts.tile([128, 64, 7], BF16)
    pp_m = consts.tile([128, 64, 7], BF16)
    stack4 = consts.tile([128, 4], F32)           # [sum1 ct0/1 | max1 ct0/1]
    Um1 = consts.tile([128, 2, 4], BF16)          # softmaxed U1 (ct, k)
    Um23 = consts.tile([64, 2, 4], BF16)          # softmaxed U2/U3 (m, k)
    U1T = consts.tile([4, 2, 128], BF16)
    UWH = consts.tile([4, 128], BF16)             # 0:64=U2T(w) 64:128=U3T(h)
    G5 = consts.tile([5, HW], BF16)               # G[r,hw]; row4 = ones
    nc.sync.dma_start(G5[4:5, :], onesd[:])
    MT5 = consts.tile([5, 256], BF16)             # (Wr U1 lam)^T; row4 = br
    nc.sync.dma_start(MT5[4:5, :], br_d[:])
    MT5s = consts.tile([5, 256], BF16)            # MT5 * spectral (folded)
    gag = consts.tile([128, 4], BF16)             # [ga ct0/1 | gm ct0/1]
    spectral = consts.tile([128, 2], F32)
    mxr2 = consts.tile([64, 1], F32)
    mxr3 = consts.tile([64, 1], F32)
    utm1 = consts.tile([128, 2, 4], F32)
    utm23 = consts.tile([64, 2, 4], F32)

    # ---- blend precompute on DVE (runs during early conv) ----
    for c2 in range(2):
        nc.vector.tensor_scalar(
            t2[:, c2].rearrange("p (h w) -> p h w", h=64),
            xr[:, 2 + c2, 1:65, 1:65], float(1.0 - alpha), None,
            op0=ALU.mult)
        nc.vector.scalar_tensor_tensor(
            Pd[:, c2].rearrange("p (h w) -> p h w", h=64),
            xr[:, c2, 1:65, 1:65], float(alpha),
            t2[:, c2].rearrange("p (h w) -> p h w", h=64),
            op0=ALU.mult, op1=ALU.subtract)

    def htree(blk, nh, slot):
        """Pairwise-add/max tree over the h axis of blk [128, nh, 64],
        writing pp_s/pp_m[:, :, slot]."""
        for op, pp in ((ALU.add, pp_s), (ALU.max, pp_m)):
            nm = "s" if op == ALU.add else "m"
            cur = blk
            n = nh
            while n > 2:
                nxt = ew.tile([128, n // 2, 64], BF16, tag=f"ht{nm}{n}")
                nc.vector.tensor_tensor(nxt[:], cur[:, 0:n // 2],
                                        cur[:, n // 2:n], op=op)
                cur = nxt
                n //= 2
            nc.vector.tensor_tensor(pp[:, :, slot], cur[:, 0], cur[:, 1],
                                    op=op)

    # batched softmax over k; exp via 4th-order Taylor (|u| ~ 0.1)
    def softmax4(ut, Uo, p, tagp, w=2):
        h1 = ew.tile([p, w, 4], F32, tag=f"h1{tagp}")
        h2 = ew.tile([p, w, 4], F32, tag=f"h2{tagp}")
        nc.vector.tensor_scalar(h1[:], ut[:], 0.25, 1.0, op0=ALU.mult,
                                op1=ALU.add)
        nc.vector.tensor_tensor(h2[:], h1[:], ut[:], op=ALU.mult)
        nc.vector.tensor_scalar(h1[:], h2[:], 1.0 / 3.0, 1.0,
                                op0=ALU.mult, op1=ALU.add)
        nc.vector.tensor_tensor(h2[:], h1[:], ut[:], op=ALU.mult)
        nc.vector.tensor_scalar(h1[:], h2[:], 0.5, 1.0, op0=ALU.mult,
                                op1=ALU.add)
        nc.vector.tensor_tensor(h2[:], h1[:], ut[:], op=ALU.mult)
        nc.vector.tensor_scalar(h1[:], h2[:], 1.0, 1.0, op0=ALU.mult,
                                op1=ALU.add)
        ssum = ew.tile([p, w], F32, tag=f"ss{tagp}")
        nc.vector.tensor_reduce(ssum[:], h1[:], axis=AX.X, op=ALU.add)
        rcp = ew.tile([p, w], F32, tag=f"rc{tagp}")
        nc.vector.reciprocal(rcp[:], ssum[:])
        nc.vector.tensor_tensor(Uo[:], h1[:],
                                rcp[:, :, None].broadcast_to([p, w, 4]),
                                op=ALU.mult)


    # ================= conv3x3 (PE) + streaming stats =================
    CHUNKS = [(0, 0), (1, 0), (0, 1), (1, 1)]     # (ct, half); h0 halves first
    with tc.tile_pool(name="ps_conv", bufs=8, space="PSUM") as ps_conv:
        for ci, (ct, hf) in enumerate(CHUNKS):
            if ci < 3:
                pss = [ps_conv.tile([128, 512], F32, tag="conv",
                                    name=f"cv{ci}{p}") for p in range(4)]
                idx = 0
                for kt in range(4):
                    for t in range(9):
                        dy, dx = t // 3, t % 3
                        for pq in range(4):
                            r0 = hf * 32 + pq * 8 + dy
                            nc.tensor.matmul(
                                pss[pq][:],
                                w3t_r[:, kt, t, ct * 128:(ct + 1) * 128],
                                xr[:, kt, r0: r0 + 8, dx: dx + 64],
                                start=(idx == 0), stop=(idx == 35))
                        idx += 1
                for pq in range(4):
                    o0 = hf * 2048 + pq * 512
                    nc.scalar.activation(Fm[:, ct, o0:o0 + 512], pss[pq][:],
                                         AF.Prelu, bias=b3_sb[:, ct:ct + 1],
                                         alpha=0.01,
                                         accum_out=sums1[:, ct, hf, pq:pq + 1])
                fm_sl = Fm[:, ct, hf * 2048:(hf + 1) * 2048]
                blk = fm_sl.rearrange("p (h w) -> p h w", h=32)
                with nc.allow_low_precision(reason="bf16 pooled partials; "
                                            "0.4% fine for attention gates"):
                    nc.vector.tensor_reduce(S3s[:, ct, hf * 32:(hf + 1) * 32],
                                            blk, axis=AX.X, op=ALU.add)
                nc.vector.tensor_reduce(S3m[:, ct, hf * 32:(hf + 1) * 32],
                                        blk, axis=AX.X, op=ALU.max)
                htree(blk, 32, ct * 2 + hf)
            else:
                for pq in range(4):
                    ps = ps_conv.tile([128, 512], F32, tag="conv",
                                      name=f"cv3{pq}")
                    idx = 0
                    for kt in range(4):
                        for t in range(9):
                            dy, dx = t // 3, t % 3
                            r0 = hf * 32 + pq * 8 + dy
                            nc.tensor.matmul(
                                ps[:],
                                w3t_r[:, kt, t, ct * 128:(ct + 1) * 128],
                                xr[:, kt, r0: r0 + 8, dx: dx + 64],
                                start=(idx == 0), stop=(idx == 35))
                            idx += 1
                    o0 = hf * 2048 + pq * 512
                    nc.scalar.activation(Fm[:, ct, o0:o0 + 512], ps[:],
                                         AF.Prelu, bias=b3_sb[:, ct:ct + 1],
                                         alpha=0.01,
                                         accum_out=sums1[:, ct, hf, pq:pq + 1])
                    sl = Fm[:, ct, o0:o0 + 512]
                    blk = sl.rearrange("p (h w) -> p h w", h=8)
                    s0 = hf * 32 + pq * 8
                    with nc.allow_low_precision(reason="bf16 pooled "
                                                "partials; 0.4% fine"):
                        nc.vector.tensor_reduce(S3s[:, ct, s0:s0 + 8], blk,
                                                axis=AX.X, op=ALU.add)
                    nc.vector.tensor_reduce(S3m[:, ct, s0:s0 + 8], blk,
                                            axis=AX.X, op=ALU.max)
                    htree(blk, 8, 3 + pq)

    # ================= pooled-stat columns -> U factors =================
    sigp = ctx.enter_context(tc.tile_pool(name="sigp", bufs=2))
    tailp = ctx.enter_context(tc.tile_pool(name="tailp", bufs=3))
    ps_sp = ctx.enter_context(tc.tile_pool(name="ps_sp", bufs=2,
                                           space="PSUM"))
    sps = []

    def spat_q(q):
        sp_ps = ps_sp.tile([128, 1024], F32, tag="sp", name=f"sp{q}")
        for hx in range(2):
            h0 = q * 1024 + hx * 512
            nc.tensor.matmul(sp_ps[:, hx * 512:(hx + 1) * 512],
                             ones4b[:], G5[0:4, h0:h0 + 512],
                             start=True, stop=True)
        sps.append(sp_ps)

    sigs = []

    def sig_q(q):
        sig = sigp.tile([128, 1024], BF16, tag="sig", name=f"sig{q}")
        nc.scalar.activation(sig[:], sps[q][:], AF.Sigmoid,
                             scale=float(ws), bias=float(bs))
        sigs.append(sig)

    def g5s_q(q):
        qs = slice(q * 1024, (q + 1) * 1024)
        nc.vector.tensor_tensor(G5s[:, qs], G5[:, qs], sigs[q][0:5, :],
                                op=ALU.mult)

    with tc.tile_pool(name="ps_mid", bufs=1, space="PSUM") as ps_mid:
        sm = ps_mid.tile([128, 512], F32, tag="sm")
        tpz = ps_mid.tile([64, 6, 128], BF16, tag="tpz")
        t3 = tpz[0:4, 0:3, :]
        mt2 = ps_mid.tile([4, 2, 128], F32, tag="mt2")
        tp = tpz[:, 3:5, :]
        avgc = sm[0:64, 26:28]
        spz = ps_mid.tile([5, 2, 256], F32, tag="spz")
        sprow = spz[0:1, 0, :]
        spec5 = spz[:, 1, :]

        # preload the sigmoid ACT table while the ACT queue is idle
        warm1 = ew.tile([1, 1], F32, tag="warm1")
        nc.scalar.activation(warm1[:], ones128[0:1, 0:1], AF.Sigmoid)

        # mode1 columns: sums + maxes per ct
        nc.vector.tensor_reduce(stack4[:, 0:2],
                                sums1[:].rearrange("p a b c -> p a (b c)"),
                                axis=AX.X, op=ALU.add)
        nc.vector.tensor_reduce(stack4[:, 2:4], S3m[:], axis=AX.X, op=ALU.max)
        # mode2 max column: combine slots, transpose, reduce
        pmx = ew.tile([128, 64], BF16, tag="pmx")
        nc.vector.tensor_reduce(pmx[:], pp_m[:], axis=AX.X, op=ALU.max)
        nc.tensor.transpose(tp[:, 0, :], pmx[:], identb[:])
        nc.vector.tensor_reduce(mxr2[:], tp[:, 0, :], axis=AX.X, op=ALU.max)
        # mode3 max column
        qmx = ew.tile([128, 64], BF16, tag="qmx")
        nc.vector.tensor_tensor(qmx[:], S3m[:, 0, :], S3m[:, 1, :],
                                op=ALU.max)
        nc.tensor.transpose(tp[:, 1, :], qmx[:], identb[:])
        nc.vector.tensor_reduce(mxr3[:], tp[:, 1, :], axis=AX.X, op=ALU.max)
        # mode2/3 avg columns via cross-partition matmuls
        for s in range(7):
            nc.tensor.matmul(avgc[:, 0:1], pp_s[:, :, s], ones128b[:],
                             start=(s == 0), stop=(s == 6))
        for cti in range(2):
            nc.tensor.matmul(avgc[:, 1:2], S3s[:, cti, :], ones128b[:],
                             start=(cti == 0), stop=(cti == 1))

        # u columns: u = wk0*avg + wk1*max + bk  (adapters folded on host)
        hlf = ew.tile([128, 2, 4], F32, tag="hlf")
        for cti in range(2):
            nc.vector.scalar_tensor_tensor(
                hlf[:, cti, :], wkb[:, 0, 1, :], stack4[:, 2 + cti:3 + cti],
                wkb[:, 0, 2, :], op0=ALU.mult, op1=ALU.add)
            nc.vector.scalar_tensor_tensor(
                utm1[:, cti, :], wkb[:, 0, 0, :], stack4[:, cti:cti + 1],
                hlf[:, cti, :], op0=ALU.mult, op1=ALU.add)
        hlf2 = ew.tile([64, 2, 4], F32, tag="hlf2")
        for mi, mxc in ((0, mxr2), (1, mxr3)):
            nc.vector.scalar_tensor_tensor(
                hlf2[:, mi, :], wkb[0:64, 1 + mi, 1, :], mxc[:],
                wkb[0:64, 1 + mi, 2, :], op0=ALU.mult, op1=ALU.add)
            nc.vector.scalar_tensor_tensor(
                utm23[:, mi, :], wkb[0:64, 1 + mi, 0, :],
                avgc[:, mi:mi + 1], hlf2[:, mi, :],
                op0=ALU.mult, op1=ALU.add)

        softmax4(utm1, Um1, 128, "a")
        softmax4(utm23, Um23, 64, "b")

        # transpose U columns to rank-rows
        for cti in range(2):
            nc.tensor.transpose(t3[:, cti, :], Um1[:, cti, :], identb[:])
        nc.tensor.transpose(t3[:, 2, 0:64], Um23[:, 0, :],
                            identb[0:64, 0:64])
        nc.tensor.transpose(t3[:, 2, 64:128], Um23[:, 1, :],
                            identb[0:64, 0:64])
        nc.scalar.copy(U1T[:], t3[:, 0:2, :])
        nc.scalar.copy(UWH[:], t3[:, 2, :])

        # spectral input stats: F_spe = U1 @ [U2;U3]^T, then mean/max
        for cti in range(2):
            fps = sm[:, 128 + cti * 128:256 + cti * 128]
            nc.tensor.matmul(fps, U1T[:, cti, :], UWH[:], start=True,
                             stop=True)
            with nc.allow_low_precision(reason="128-val mean into bf16 for "
                                        "double-sigmoid gate; 0.4% is fine"):
                nc.vector.tensor_reduce(gag[:, cti:cti + 1], fps, axis=AX.X,
                                        op=ALU.add)
            nc.vector.tensor_reduce(gag[:, 2 + cti:3 + cti], fps, axis=AX.X,
                                    op=ALU.max)

        # G5 construction + first spatial/sig tiles: only need UWH;
        # overlap the spectral/MT chains below on other engines
        def g5_chunk(q):
            qs = slice(q * 1024, (q + 1) * 1024)
            nc.vector.tensor_tensor(
                G5[0:4, qs].rearrange("p (h w) -> p h w", h=16),
                UWH[:, 64 + q * 16:64 + (q + 1) * 16][:, :, None]
                .broadcast_to([4, 16, 64]),
                UWH[:, 0:64][:, None, :].broadcast_to([4, 16, 64]),
                op=ALU.mult)

        g5_chunk(0)
        g5_chunk(1)
        g5_chunk(2)
        g5_chunk(3)
        spat_q(0)
        spat_q(1)
        sig_q(0)
        sig_q(1)
        g5s_q(0)
        g5s_q(1)

        # spectral attention (double sigmoid)
        for mm in range(2):
            sp_ps = sm[:, 24 + mm:25 + mm]
            for kk in range(4):
                nc.tensor.matmul(sp_ps, wsc_b[:, kk, mm, :],
                                 gag[:, kk:kk + 1], start=(kk == 0),
                                 stop=(kk == 3))
            stmp = ew.tile([128, 1], F32, tag=f"stmp{mm}")
            nc.scalar.activation(stmp[:], sp_ps, AF.Sigmoid,
                                 bias=bsc_sb[:, mm:mm + 1])
            nc.scalar.activation(spectral[:, mm:mm + 1], stmp[:], AF.Sigmoid)


        # MT = (Wr @ U1 diag(lam))^T, then fold spectral into columns
        for mm in range(2):
            m_ps = sm[:, 16 + mm * 4:20 + mm * 4]
            for kk in range(2):
                nc.tensor.matmul(m_ps, wrt_b[:, kk, mm, :], Um1[:, kk, :],
                                 start=(kk == 0), stop=(kk == 1))
            m_sb = ew.tile([128, 4], F32, tag=f"msb{mm}")
            nc.scalar.copy(m_sb[:], m_ps)
            nc.tensor.transpose(mt2[:, mm, :], m_sb[:], ident[:])
            with nc.allow_low_precision(reason="bf16 CP factors; gates "
                                        "tolerate 0.4%"):
                nc.vector.tensor_scalar(MT5[0:4, mm * 128:(mm + 1) * 128],
                                        mt2[:, mm, :], lam_sb[:], None,
                                        op0=ALU.mult)
        nc.tensor.transpose(sprow[0:1, 0:128], spectral[:, 0:1], ident[:])
        nc.tensor.transpose(sprow[0:1, 128:256], spectral[:, 1:2], ident[:])
        sprow_sb = ew.tile([1, 256], BF16, tag="sprow_sb")
        nc.scalar.copy(sprow_sb[:], sprow[:])
        nc.tensor.matmul(spec5[:], ones8[0:1, 0:5], sprow_sb[:],
                         start=True, stop=True)
        with nc.allow_low_precision(reason="bf16 recon factors; 0.4% fine"):
            nc.vector.tensor_tensor(MT5s[:], MT5[:], spec5[:], op=ALU.mult)

    # ================= final elementwise stage =================
    # fused = (sig*spec)*Pd + t2 ; cp_recon = (MT5s G)*sig + Fm
    with tc.tile_pool(name="ps_tail", bufs=2, space="PSUM") as ps_tail:
        def pds_prep():
            # PdS = Pd * spectral (per-partition scale; tensor_scalar 4x)
            for ct in range(2):
                nc.vector.tensor_scalar(PdS[:, ct, :], Pd[:, ct, :],
                                        spectral[:, ct:ct + 1], None,
                                        op0=ALU.mult)

        def tail_q(q):
            sig = sigs[q]
            sigv = sig[:].rearrange("p (h w) -> p h w", h=16)
            qs = slice(q * 1024, (q + 1) * 1024)
            for ct in range(2):
                # rc = (MT5s @ G5s) + Fm  entirely in PSUM (PE-only cpr)
                rc = ps_tail.tile([128, 1024], F32, tag="rc",
                                  name=f"rc{q}{ct}")
                for hx in range(2):
                    h0 = q * 1024 + hx * 512
                    nc.tensor.matmul(rc[:, hx * 512:(hx + 1) * 512],
                                     MT5s[:, ct * 128:(ct + 1) * 128],
                                     G5s[:, h0:h0 + 512], start=True,
                                     stop=False)
                    nc.tensor.matmul(rc[:, hx * 512:(hx + 1) * 512],
                                     identb[:], Fm[:, ct, h0:h0 + 512],
                                     start=False, stop=True)
                rcb = sigp.tile([128, 1024], BF16, tag="rcb",
                                name=f"rcb{q}{ct}")
                nc.scalar.copy(rcb[:], rc[:])
                nc.gpsimd.dma_start(cpr_o[ct, :, q * 16:(q + 1) * 16, :],
                                    rcb[:].rearrange("p (h w) -> p h w",
                                                     h=16))
                A2 = tailp.tile([128, 16, 64], BF16, tag="A2",
                                name=f"A2{q}{ct}")
                nc.vector.tensor_tensor(
                    A2[:], PdS[:, ct, qs].rearrange("p (h w) -> p h w", h=16),
                    sigv, op=ALU.mult)
                fu = tailp.tile([128, 16, 64], BF16, tag="fu",
                                name=f"fu{q}{ct}")
                nc.vector.tensor_tensor(
                    fu[:], A2[:],
                    t2[:, ct, qs].rearrange("p (h w) -> p h w", h=16),
                    op=ALU.add)
                nc.sync.dma_start(fused_o[ct, :, q * 16:(q + 1) * 16, :],
                                  fu[:])

        pds_prep()
        tail_q(0)
        spat_q(2)
        sig_q(2)
        g5s_q(2)
        tail_q(1)
        spat_q(3)
        sig_q(3)
        g5s_q(3)
        tail_q(2)
        tail_q(3)
    ctx.close()


def _prep_weights(W3, b3, Wa1, ba1, Wa2, ba2, Wa3, ba3, Wu, bu, Wr, br,
                  Wsa, bsa, Wsm, bsm):
    f = np.float32
    # w3t[p, kt, t, co] = W3[co, kt*128+p, dy, dx]
    w3t = np.ascontiguousarray(
        W3.reshape(C, 4, 128, 9).transpose(2, 1, 3, 0)).astype(f)
    b3h = np.ascontiguousarray(b3.reshape(2, 128).T).astype(f)
    # adapter + U_gen collapsed: u = (Wu@Wa)@[avg;max] + (Wu@ba + bu)
    wkb = np.zeros((3, 3, 4), f)
    for mi, (Wa, ba, div) in enumerate(
            ((Wa1, ba1, HW), (Wa2, ba2, C * H), (Wa3, ba3, C * W))):
        wk = Wu @ Wa                               # [4, 2]
        wkb[mi, 0] = wk[:, 0] / float(div)
        wkb[mi, 1] = wk[:, 1]
        wkb[mi, 2] = Wu @ ba + bu
    wkb = np.ascontiguousarray(
        np.broadcast_to(wkb[None], (128, 3, 3, 4))).astype(f)
    # wrt[p, kk, mm, m] = Wr[mm*128+m, kk*128+p]
    wrt = np.ascontiguousarray(
        Wr.reshape(2, 128, 2, 128).transpose(3, 2, 0, 1)).astype(f)
    brh = br.reshape(1, 256).astype(f)
    # wsc[p, kk, mm, m]: kk<2 -> Wsa/128 (mean folded), kk>=2 -> Wsm
    wsa_r = (Wsa / 128.0).reshape(2, 128, 2, 128).transpose(3, 2, 0, 1)
    wsm_r = Wsm.reshape(2, 128, 2, 128).transpose(3, 2, 0, 1)
    wsc = np.ascontiguousarray(
        np.concatenate([wsa_r, wsm_r], axis=1)).astype(f)
    bsc = np.ascontiguousarray((bsa + bsm).reshape(2, 128).T).astype(f)
    import ml_dtypes
    bf = ml_dtypes.bfloat16
    return dict(w3t=w3t.astype(bf), bb=np.ascontiguousarray(
                    np.concatenate([b3h, bsc], axis=1)),
                wkb=wkb, wrt=wrt.astype(bf), br=brh.astype(bf),
                wsc=wsc.astype(bf), onesd=np.ones((1, HW), bf))


_CACHE = {}


def kernel(frm_feat, other_feat, W3, b3, Wa1, ba1, Wa2, ba2, Wa3, ba3,
           Wu, bu, Wr, br, ws, bs, Wsa, bsa, Wsm, bsm, alpha, lam,
           _trace=False, _tmpdir=None):
    import ml_dtypes
    bf = ml_dtypes.bfloat16
    frm_feat = np.asarray(frm_feat, np.float32)
    other_feat = np.asarray(other_feat, np.float32)
    key = (float(alpha), float(ws), float(bs))
    if key not in _CACHE:
        _CACHE[key] = build_program(float(alpha), float(ws), float(bs))
    nc = _CACHE[key]

    wd = _prep_weights(np.asarray(W3), np.asarray(b3), np.asarray(Wa1),
                       np.asarray(ba1), np.asarray(Wa2), np.asarray(ba2),
                       np.asarray(Wa3), np.asarray(ba3), np.asarray(Wu),
                       np.asarray(bu), np.asarray(Wr), np.asarray(br),
                       np.asarray(Wsa), np.asarray(bsa), np.asarray(Wsm),
                       np.asarray(bsm))
    wd["lam"] = np.asarray(lam, np.float32).reshape(4, 1)

    in_maps = []
    for b_i in range(NCORES):
        m = dict(wd)
        xin = np.zeros((128, 4, 66, 66), bf)
        xin[:, 0:2, 1:65, 1:65] = frm_feat[b_i].reshape(
            2, 128, 64, 64).transpose(1, 0, 2, 3).astype(bf)
        xin[:, 2:4, 1:65, 1:65] = other_feat[b_i].reshape(
            2, 128, 64, 64).transpose(1, 0, 2, 3).astype(bf)
        m["xin"] = xin
        in_maps.append(m)

    res = bass_utils.run_bass_kernel_spmd(
        nc, in_maps, core_ids=list(range(NCORES)), trace=_trace,
        tmpdir=_tmpdir)
    fused = np.stack([
        np.asarray(res.results[i]["fused"]).astype(np.float32).reshape(C, H, W)
        for i in range(NCORES)])
    cpr = np.stack([
        np.asarray(res.results[i]["cpr"]).astype(np.float32).reshape(C, H, W)
        for i in range(NCORES)])
    kernel._last_exec_time_ns = res.exec_time_ns
    kernel._last_results = res
    return fused, cpr


# revision 37
# speedup vs baseline: 1.0128x; 1.0019x over previous
"""Trainium2 Bass kernel for nn_MDRMWithCPRecon.

Sharding: pure data parallel over batch B=8 -> one batch element per
NeuronCore (8 cores). All parameters replicated. Each core computes the
full per-batch pipeline:

  x = cat(frm, oth)                 [512, 64, 64]
  Fm = lrelu(conv3x3(x, W3) + b3)   [256, 64, 64]   <- bulk of FLOPs
  U1/U2/U3 rank-4 softmax factors from pooled stats (tiny ops)
  spatial  = sigmoid(ws * U3 @ U2^T + bs)
  spectral = sigmoid(sigmoid(Wsa@mean + Wsm@max + biases))
  Wt = spectral x spatial
  fused    = a*Wt*frm + (1-a)*(1-Wt)*oth
  cp_recon = (Wr @ cp + br) * Wt + Fm,  cp = rank-4 CP(U1,U2,U3,lam)

Perf structure (vs the 218us f32r / 200us bf16 versions):
  - conv3x3 in bf16 (inputs + weights converted on host): 1 cyc/row
    matmul rate with fast-weight-load; input DMA bytes halved vs f32.
  - conv as 9-tap PSUM-accumulated matmuls, weight-major: each weight
    load feeds 4 consecutive matmuls into 4 parallel PSUM accumulators.
  - h-pooled stats via pairwise TT trees (unit-stride bf16, 2x DVE)
    instead of strided-view reduces.
  - adapter+U_gen collapsed on host: u = (Wu@Wa)@[avg;max] + (Wu@ba+bu),
    so pooled COLUMNS feed tiny scalar_tensor_tensor ops directly; no
    row transposes / 384-col adapter matmuls.
  - spectral folded into MT5 columns and into Pd once (tensor_scalar
    runs 4x on DVE; scalar_tensor_tensor measured 1x on HW - avoid).
  - sig folded into G5 rows (G5s) so the whole cp_recon path is PE
    matmul accumulation (MT5s @ G5s + I @ Fm) + one ACT copy per tile;
    plain DMA stores (no DMA-accumulate: its software-DGE descriptor
    flood cost ~18us of post-compute drain in the 218us version).
  - G5 outer product built in 4 chunks; spatial/sig tiles pipelined
    against the spectral/MT chains.
  - outputs stored bf16 and widened to f32 on the host.
"""

import numpy as np

import concourse.bacc as bacc
import concourse.bass as bass
import concourse.tile as tile
from concourse import mybir, bass_utils

F32 = mybir.dt.float32
BF16 = mybir.dt.bfloat16
AF = mybir.ActivationFunctionType
ALU = mybir.AluOpType
AX = mybir.AxisListType

B, C, H, W, K = 8, 256, 64, 64, 4
HW = H * W
NCORES = 8


def build_program(alpha, ws, bs):
    from concourse.masks import make_identity

    nc = bacc.Bacc("TRN2", target_bir_lowering=False, debug=False,
                   num_devices=NCORES)

    # ---- DRAM I/O (per core) ----
    xin_d = nc.dram_tensor("xin", [128, 4, 66, 66], BF16,
                           kind="ExternalInput")
    w3t_d = nc.dram_tensor("w3t", [128, 4, 9, 256], BF16,
                           kind="ExternalInput")
    bb_d = nc.dram_tensor("bb", [128, 4], F32, kind="ExternalInput")
    wkb_d = nc.dram_tensor("wkb", [128, 3, 3, 4], F32, kind="ExternalInput")
    wrt_d = nc.dram_tensor("wrt", [128, 2, 2, 128], BF16,
                           kind="ExternalInput")
    br_d = nc.dram_tensor("br", [1, 256], BF16, kind="ExternalInput")
    wsc_d = nc.dram_tensor("wsc", [128, 4, 2, 128], BF16,
                           kind="ExternalInput")
    lam_d = nc.dram_tensor("lam", [4, 1], F32, kind="ExternalInput")
    onesd = nc.dram_tensor("onesd", [1, HW], BF16, kind="ExternalInput")
    fused_o = nc.dram_tensor("fused", [2, 128, H, W], BF16,
                             kind="ExternalOutput")
    cpr_o = nc.dram_tensor("cpr", [2, 128, H, W], BF16, kind="ExternalOutput")

    with tile.TileContext(nc) as tc:
        _build_tile(tc, nc, make_identity, locals(), alpha, ws, bs)
    nc.compile()
    return nc


def _build_tile(tc, nc, make_identity, T, alpha, ws, bs):
    xin_d, w3t_d = T["xin_d"], T["w3t_d"]
    bb_d, wkb_d = T["bb_d"], T["wkb_d"]
    wrt_d, br_d, wsc_d, lam_d = T["wrt_d"], T["br_d"], T["wsc_d"], T["lam_d"]
    onesd = T["onesd"]
    fused_o, cpr_o = T["fused_o"], T["cpr_o"]

    import contextlib
    ctx = contextlib.ExitStack()
    consts = ctx.enter_context(tc.tile_pool(name="consts", bufs=1))
    stage = ctx.enter_context(tc.tile_pool(name="stage", bufs=2))
    ew = stage

    # conv weights + host-padded image in bf16 (contiguous DMAs)
    w3t_r = consts.tile([128, 4, 9, 256], BF16)
    xr = consts.tile([128, 4, 66, 66], BF16)

    # ---- input DMAs first: small kt0 gating pieces, then the rest ----
    nc.sync.dma_start(w3t_r[:, 0, 0:3], w3t_d[:, 0, 0:3])
    nc.sync.dma_start(xr[:, 0, 0:12, :], xin_d[:, 0, 0:12, :])
    nc.sync.dma_start(xr[:, 0, 12:23, :], xin_d[:, 0, 12:23, :])
    nc.sync.dma_start(w3t_r[:, 0, 3:9], w3t_d[:, 0, 3:9])
    nc.sync.dma_start(xr[:, 0, 23:34, :], xin_d[:, 0, 23:34, :])
    for kt in range(1, 4):
        nc.sync.dma_start(w3t_r[:, kt], w3t_d[:, kt])
        nc.sync.dma_start(xr[:, kt, 0:34, :], xin_d[:, kt, 0:34, :])
    for kt in range(4):
        nc.sync.dma_start(xr[:, kt, 34:66, :], xin_d[:, kt, 34:66, :])

    # ================= consts / layout =================
    ident = consts.tile([128, 128], F32)
    make_identity(nc, ident[:])
    identb = consts.tile([128, 128], BF16)
    nc.vector.tensor_copy(identb[:], ident[:])
    ones128 = consts.tile([128, 1], F32)
    nc.gpsimd.memset(ones128[:], 1.0)
    ones128b = consts.tile([128, 1], BF16)
    nc.gpsimd.memset(ones128b[:], 1.0)
    ones8 = consts.tile([1, 8], BF16)
    nc.gpsimd.memset(ones8[:], 1.0)
    ones4b = consts.tile([4, 128], BF16)
    nc.gpsimd.memset(ones4b[:], 1.0)

    # ---- small weights ----
    bb_sb = consts.tile([128, 4], F32)
    nc.gpsimd.dma_start(bb_sb[:], bb_d[:])
    lam_sb = consts.tile([4, 1], F32)
    nc.gpsimd.dma_start(lam_sb[:], lam_d[:])
    wkb = consts.tile([128, 3, 3, 4], F32)
    nc.gpsimd.dma_start(wkb[:], wkb_d[:])
    wrt_b = consts.tile([128, 2, 2, 128], BF16)
    nc.gpsimd.dma_start(wrt_b[:], wrt_d[:])
    wsc_b = consts.tile([128, 4, 2, 128], BF16)
    nc.gpsimd.dma_start(wsc_b[:], wsc_d[:])
    b3_sb = bb_sb[:, 0:2]                          # f32 conv bias
    bsc_sb = bb_sb[:, 2:4]                         # f32 spectral bias

    # ---- persistent intermediates ----
    Fm = consts.tile([128, 2, HW], BF16)          # conv output, (ct, h*64+w)
    Pd = consts.tile([128, 2, HW], BF16)          # alpha*frm - (1-alpha)*oth
    PdS = consts.tile([128, 2, HW], BF16)         # Pd * spectral (tail)
    G5s = consts.tile([5, HW], BF16)              # G5 * sig (tail)
    t2 = consts.tile([128, 2, HW], BF16)          # (1-alpha)*oth
    sums1 = consts.tile([128, 2, 2, 4], F32)      # ACT accum per 512-tile
    S3s = consts.tile([128, 2, 64], BF16)         # mode3 h-sums per ct
    S3m = consts.tile([128, 2, 64], BF16)         # mode3 h-maxes per ct
    # mode2 partials, w-major w/ slot innermost: 0,1=ct0 2=ct1h0 3:7=ct1h1
    pp_s = consts.tile([128, 64, 7], BF16)
    pp_m = consts.tile([128, 64, 7], BF16)
    stack4 = consts.tile([128, 4], F32)           # [sum1 ct0/1 | max1 ct0/1]
    Um1 = consts.tile([128, 2, 4], BF16)          # softmaxed U1 (ct, k)
    Um23 = consts.tile([64, 2, 4], BF16)          # softmaxed U2/U3 (m, k)
    U1T = consts.tile([4, 2, 128], BF16)
    UWH = consts.tile([4, 128], BF16)             # 0:64=U2T(w) 64:128=U3T(h)
    G5 = consts.tile([5, HW], BF16)               # G[r,hw]; row4 = ones
    nc.sync.dma_start(G5[4:5, :], onesd[:])
    MT5 = consts.tile([5, 256], BF16)             # (Wr U1 lam)^T; row4 = br
    nc.sync.dma_start(MT5[4:5, :], br_d[:])
    MT5s = consts.tile([5, 256], BF16)            # MT5 * spectral (folded)
    gag = consts.tile([128, 4], BF16)             # [ga ct0/1 | gm ct0/1]
    spectral = consts.tile([128, 2], F32)
    mxr2 = consts.tile([64, 1], F32)
    mxr3 = consts.tile([64, 1], F32)
    utm1 = consts.tile([128, 2, 4], F32)
    utm23 = consts.tile([64, 2, 4], F32)

    # ---- blend precompute on DVE (runs during early conv) ----
    for c2 in range(2):
        nc.vector.tensor_scalar(
            t2[:, c2].rearrange("p (h w) -> p h w", h=64),
            xr[:, 2 + c2, 1:65, 1:65], float(1.0 - alpha), None,
            op0=ALU.mult)
        nc.vector.scalar_tensor_tensor(
            Pd[:, c2].rearrange("p (h w) -> p h w", h=64),
            xr[:, c2, 1:65, 1:65], float(alpha),
            t2[:, c2].rearrange("p (h w) -> p h w", h=64),
            op0=ALU.mult, op1=ALU.subtract)

    def htree(blk, nh, slot):
        """Pairwise-add/max tree over the h axis of blk [128, nh, 64],
        writing pp_s/pp_m[:, :, slot]."""
        for op, pp in ((ALU.add, pp_s), (ALU.max, pp_m)):
            nm = "s" if op == ALU.add else "m"
            cur = blk
            n = nh
            while n > 2:
                nxt = ew.tile([128, n // 2, 64], BF16, tag=f"ht{nm}{n}")
                nc.vector.tensor_tensor(nxt[:], cur[:, 0:n // 2],
                                        cur[:, n // 2:n], op=op)
                cur = nxt
                n //= 2
            nc.vector.tensor_tensor(pp[:, :, slot], cur[:, 0], cur[:, 1],
                                    op=op)

    # batched softmax over k; exp via 4th-order Taylor (|u| ~ 0.1)
    def softmax4(ut, Uo, p, tagp, w=2):
        h1 = ew.tile([p, w, 4], F32, tag=f"h1{tagp}")
        h2 = ew.tile([p, w, 4], F32, tag=f"h2{tagp}")
        nc.vector.tensor_scalar(h1[:], ut[:], 0.25, 1.0, op0=ALU.mult,
                                op1=ALU.add)
        nc.vector.tensor_tensor(h2[:], h1[:], ut[:], op=ALU.mult)
        nc.vector.tensor_scalar(h1[:], h2[:], 1.0 / 3.0, 1.0,
                                op0=ALU.mult, op1=ALU.add)
        nc.vector.tensor_tensor(h2[:], h1[:], ut[:], op=ALU.mult)
        nc.vector.tensor_scalar(h1[:], h2[:], 0.5, 1.0, op0=ALU.mult,
                                op1=ALU.add)
        nc.vector.tensor_tensor(h2[:], h1[:], ut[:], op=ALU.mult)
        nc.vector.tensor_scalar(h1[:], h2[:], 1.0, 1.0, op0=ALU.mult,
                                op1=ALU.add)
        ssum = ew.tile([p, w], F32, tag=f"ss{tagp}")
        nc.vector.tensor_reduce(ssum[:], h1[:], axis=AX.X, op=ALU.add)
        rcp = ew.tile([p, w], F32, tag=f"rc{tagp}")
        nc.vector.reciprocal(rcp[:], ssum[:])
        nc.vector.tensor_tensor(Uo[:], h1[:],
                                rcp[:, :, None].broadcast_to([p, w, 4]),
                                op=ALU.mult)


    # ================= conv3x3 (PE) + streaming stats =================
    CHUNKS = [(0, 0), (1, 0), (0, 1), (1, 1)]     # (ct, half); h0 halves first
    with tc.tile_pool(name="ps_conv", bufs=8, space="PSUM") as ps_conv:
        for ci, (ct, hf) in enumerate(CHUNKS):
            if ci < 3:
                pss = [ps_conv.tile([128, 512], F32, tag="conv",
                                    name=f"cv{ci}{p}") for p in range(4)]
                idx = 0
                for kt in range(4):
                    for t in range(9):
                        dy, dx = t // 3, t % 3
                        for pq in range(4):
                            r0 = hf * 32 + pq * 8 + dy
                            nc.tensor.matmul(
                                pss[pq][:],
                                w3t_r[:, kt, t, ct * 128:(ct + 1) * 128],
                                xr[:, kt, r0: r0 + 8, dx: dx + 64],
                                start=(idx == 0), stop=(idx == 35))
                        idx += 1
                for pq in range(4):
                    o0 = hf * 2048 + pq * 512
                    nc.scalar.activation(Fm[:, ct, o0:o0 + 512], pss[pq][:],
                                         AF.Prelu, bias=b3_sb[:, ct:ct + 1],
                                         alpha=0.01,
                                         accum_out=sums1[:, ct, hf, pq:pq + 1])
                fm_sl = Fm[:, ct, hf * 2048:(hf + 1) * 2048]
                blk = fm_sl.rearrange("p (h w) -> p h w", h=32)
                with nc.allow_low_precision(reason="bf16 pooled partials; "
                                            "0.4% fine for attention gates"):
                    nc.vector.tensor_reduce(S3s[:, ct, hf * 32:(hf + 1) * 32],
                                            blk, axis=AX.X, op=ALU.add)
                nc.vector.tensor_reduce(S3m[:, ct, hf * 32:(hf + 1) * 32],
                                        blk, axis=AX.X, op=ALU.max)
                htree(blk, 32, ct * 2 + hf)
            else:
                for pq in range(4):
                    ps = ps_conv.tile([128, 512], F32, tag="conv",
                                      name=f"cv3{pq}")
                    idx = 0
                    for kt in range(4):
                        for t in range(9):
                            dy, dx = t // 3, t % 3
                            r0 = hf * 32 + pq * 8 + dy
                            nc.tensor.matmul(
                                ps[:],
                                w3t_r[:, kt, t, ct * 128:(ct + 1) * 128],
                                xr[:, kt, r0: r0 + 8, dx: dx + 64],
                                start=(idx == 0), stop=(idx == 35))
                            idx += 1
                    o0 = hf * 2048 + pq * 512
                    nc.scalar.activation(Fm[:, ct, o0:o0 + 512], ps[:],
                                         AF.Prelu, bias=b3_sb[:, ct:ct + 1],
                                         alpha=0.01,
                                         accum_out=sums1[:, ct, hf, pq:pq + 1])
                    sl = Fm[:, ct, o0:o0 + 512]
                    blk = sl.rearrange("p (h w) -> p h w", h=8)
                    s0 = hf * 32 + pq * 8
                    with nc.allow_low_precision(reason="bf16 pooled "
                                                "partials; 0.4% fine"):
                        nc.vector.tensor_reduce(S3s[:, ct, s0:s0 + 8], blk,
                                                axis=AX.X, op=ALU.add)
                    nc.vector.tensor_reduce(S3m[:, ct, s0:s0 + 8], blk,
                                            axis=AX.X, op=ALU.max)
                    htree(blk, 8, 3 + pq)

    # PE pstate warmers: ~96 standalone bf16 weight loads bridge the
    # post-conv gap so the tail matmuls stay at max clock (measured
    # 535-602ns vs 213ns theoretical at mid-pstate). No PSUM, no deps.
    for wi in range(96):
        nc.tensor.ldweights(w3t_r[:, wi % 4, wi % 9, 0:128])

    # ================= pooled-stat columns -> U factors =================
    sigp = ctx.enter_context(tc.tile_pool(name="sigp", bufs=2))
    tailp = ctx.enter_context(tc.tile_pool(name="tailp", bufs=3))
    ps_sp = ctx.enter_context(tc.tile_pool(name="ps_sp", bufs=2,
                                           space="PSUM"))
    sps = []

    def spat_q(q):
        sp_ps = ps_sp.tile([128, 1024], F32, tag="sp", name=f"sp{q}")
        for hx in range(2):
            h0 = q * 1024 + hx * 512
            nc.tensor.matmul(sp_ps[:, hx * 512:(hx + 1) * 512],
                             ones4b[:], G5[0:4, h0:h0 + 512],
                             start=True, stop=True)
        sps.append(sp_ps)

    sigs = []

    def sig_q(q):
        sig = sigp.tile([128, 1024], BF16, tag="sig", name=f"sig{q}")
        nc.scalar.activation(sig[:], sps[q][:], AF.Sigmoid,
                             scale=float(ws), bias=float(bs))
        sigs.append(sig)

    def g5s_q(q):
        qs = slice(q * 1024, (q + 1) * 1024)
        nc.vector.tensor_tensor(G5s[:, qs], G5[:, qs], sigs[q][0:5, :],
                                op=ALU.mult)

    with tc.tile_pool(name="ps_mid", bufs=1, space="PSUM") as ps_mid:
        sm = ps_mid.tile([128, 512], F32, tag="sm")
        tpz = ps_mid.tile([64, 6, 128], BF16, tag="tpz")
        t3 = tpz[0:4, 0:3, :]
        mt2 = ps_mid.tile([4, 2, 128], F32, tag="mt2")
        tp = tpz[:, 3:5, :]
        avgc = sm[0:64, 26:28]
        spz = ps_mid.tile([5, 2, 256], F32, tag="spz")
        sprow = spz[0:1, 0, :]
        spec5 = spz[:, 1, :]

        # preload the sigmoid ACT table while the ACT queue is idle
        warm1 = ew.tile([1, 1], F32, tag="warm1")
        nc.scalar.activation(warm1[:], ones128[0:1, 0:1], AF.Sigmoid)

        # mode1 columns: sums + maxes per ct
        nc.vector.tensor_reduce(stack4[:, 0:2],
                                sums1[:].rearrange("p a b c -> p a (b c)"),
                                axis=AX.X, op=ALU.add)
        nc.vector.tensor_reduce(stack4[:, 2:4], S3m[:], axis=AX.X, op=ALU.max)
        # mode2 max column: combine slots, transpose, reduce
        pmx = ew.tile([128, 64], BF16, tag="pmx")
        nc.vector.tensor_reduce(pmx[:], pp_m[:], axis=AX.X, op=ALU.max)
        nc.tensor.transpose(tp[:, 0, :], pmx[:], identb[:])
        nc.vector.tensor_reduce(mxr2[:], tp[:, 0, :], axis=AX.X, op=ALU.max)
        # mode3 max column
        qmx = ew.tile([128, 64], BF16, tag="qmx")
        nc.vector.tensor_tensor(qmx[:], S3m[:, 0, :], S3m[:, 1, :],
                                op=ALU.max)
        nc.tensor.transpose(tp[:, 1, :], qmx[:], identb[:])
        nc.vector.tensor_reduce(mxr3[:], tp[:, 1, :], axis=AX.X, op=ALU.max)
        # mode2/3 avg columns via cross-partition matmuls
        for s in range(7):
            nc.tensor.matmul(avgc[:, 0:1], pp_s[:, :, s], ones128b[:],
                             start=(s == 0), stop=(s == 6))
        for cti in range(2):
            nc.tensor.matmul(avgc[:, 1:2], S3s[:, cti, :], ones128b[:],
                             start=(cti == 0), stop=(cti == 1))

        # u columns: u = wk0*avg + wk1*max + bk  (adapters folded on host)
        hlf = ew.tile([128, 2, 4], F32, tag="hlf")
        for cti in range(2):
            nc.vector.scalar_tensor_tensor(
                hlf[:, cti, :], wkb[:, 0, 1, :], stack4[:, 2 + cti:3 + cti],
                wkb[:, 0, 2, :], op0=ALU.mult, op1=ALU.add)
            nc.vector.scalar_tensor_tensor(
                utm1[:, cti, :], wkb[:, 0, 0, :], stack4[:, cti:cti + 1],
                hlf[:, cti, :], op0=ALU.mult, op1=ALU.add)
        hlf2 = ew.tile([64, 2, 4], F32, tag="hlf2")
        for mi, mxc in ((0, mxr2), (1, mxr3)):
            nc.vector.scalar_tensor_tensor(
                hlf2[:, mi, :], wkb[0:64, 1 + mi, 1, :], mxc[:],
                wkb[0:64, 1 + mi, 2, :], op0=ALU.mult, op1=ALU.add)
            nc.vector.scalar_tensor_tensor(
                utm23[:, mi, :], wkb[0:64, 1 + mi, 0, :],
                avgc[:, mi:mi + 1], hlf2[:, mi, :],
                op0=ALU.mult, op1=ALU.add)

        softmax4(utm1, Um1, 128, "a")
        softmax4(utm23, Um23, 64, "b")

        # transpose U columns to rank-rows
        for cti in range(2):
            nc.tensor.transpose(t3[:, cti, :], Um1[:, cti, :], identb[:])
        nc.tensor.transpose(t3[:, 2, 0:64], Um23[:, 0, :],
                            identb[0:64, 0:64])
        nc.tensor.transpose(t3[:, 2, 64:128], Um23[:, 1, :],
                            identb[0:64, 0:64])
        nc.scalar.copy(U1T[:], t3[:, 0:2, :])
        nc.scalar.copy(UWH[:], t3[:, 2, :])

        # spectral input stats: F_spe = U1 @ [U2;U3]^T, then mean/max
        for cti in range(2):
            fps = sm[:, 128 + cti * 128:256 + cti * 128]
            nc.tensor.matmul(fps, U1T[:, cti, :], UWH[:], start=True,
                             stop=True)
            with nc.allow_low_precision(reason="128-val mean into bf16 for "
                                        "double-sigmoid gate; 0.4% is fine"):
                nc.vector.tensor_reduce(gag[:, cti:cti + 1], fps, axis=AX.X,
                                        op=ALU.add)
            nc.vector.tensor_reduce(gag[:, 2 + cti:3 + cti], fps, axis=AX.X,
                                    op=ALU.max)

        # G5 construction + first spatial/sig tiles: only need UWH;
        # overlap the spectral/MT chains below on other engines
        def g5_chunk(q):
            qs = slice(q * 1024, (q + 1) * 1024)
            nc.vector.tensor_tensor(
                G5[0:4, qs].rearrange("p (h w) -> p h w", h=16),
                UWH[:, 64 + q * 16:64 + (q + 1) * 16][:, :, None]
                .broadcast_to([4, 16, 64]),
                UWH[:, 0:64][:, None, :].broadcast_to([4, 16, 64]),
                op=ALU.mult)

        g5_chunk(0)
        g5_chunk(1)
        g5_chunk(2)
        g5_chunk(3)
        spat_q(0)
        spat_q(1)
        sig_q(0)
        sig_q(1)
        g5s_q(0)
        g5s_q(1)

        # spectral attention (double sigmoid)
        for mm in range(2):
            sp_ps = sm[:, 24 + mm:25 + mm]
            for kk in range(4):
                nc.tensor.matmul(sp_ps, wsc_b[:, kk, mm, :],
                                 gag[:, kk:kk + 1], start=(kk == 0),
                                 stop=(kk == 3))
            stmp = ew.tile([128, 1], F32, tag=f"stmp{mm}")
            nc.scalar.activation(stmp[:], sp_ps, AF.Sigmoid,
                                 bias=bsc_sb[:, mm:mm + 1])
            nc.scalar.activation(spectral[:, mm:mm + 1], stmp[:], AF.Sigmoid)


        # MT = (Wr @ U1 diag(lam))^T, then fold spectral into columns
        for mm in range(2):
            m_ps = sm[:, 16 + mm * 4:20 + mm * 4]
            for kk in range(2):
                nc.tensor.matmul(m_ps, wrt_b[:, kk, mm, :], Um1[:, kk, :],
                                 start=(kk == 0), stop=(kk == 1))
            m_sb = ew.tile([128, 4], F32, tag=f"msb{mm}")
            nc.scalar.copy(m_sb[:], m_ps)
            nc.tensor.transpose(mt2[:, mm, :], m_sb[:], ident[:])
            with nc.allow_low_precision(reason="bf16 CP factors; gates "
                                        "tolerate 0.4%"):
                nc.vector.tensor_scalar(MT5[0:4, mm * 128:(mm + 1) * 128],
                                        mt2[:, mm, :], lam_sb[:], None,
                                        op0=ALU.mult)
        nc.tensor.transpose(sprow[0:1, 0:128], spectral[:, 0:1], ident[:])
        nc.tensor.transpose(sprow[0:1, 128:256], spectral[:, 1:2], ident[:])
        sprow_sb = ew.tile([1, 256], BF16, tag="sprow_sb")
        nc.scalar.copy(sprow_sb[:], sprow[:])
        nc.tensor.matmul(spec5[:], ones8[0:1, 0:5], sprow_sb[:],
                         start=True, stop=True)
        with nc.allow_low_precision(reason="bf16 recon factors; 0.4% fine"):
            nc.vector.tensor_tensor(MT5s[:], MT5[:], spec5[:], op=ALU.mult)

    # ================= final elementwise stage =================
    # fused = (sig*spec)*Pd + t2 ; cp_recon = (MT5s G)*sig + Fm
    with tc.tile_pool(name="ps_tail", bufs=2, space="PSUM") as ps_tail:
        def pds_prep():
            # PdS = Pd * spectral (per-partition scale; tensor_scalar 4x)
            for ct in range(2):
                nc.vector.tensor_scalar(PdS[:, ct, :], Pd[:, ct, :],
                                        spectral[:, ct:ct + 1], None,
                                        op0=ALU.mult)

        def tail_q(q):
            sig = sigs[q]
            sigv = sig[:].rearrange("p (h w) -> p h w", h=16)
            qs = slice(q * 1024, (q + 1) * 1024)
            for ct in range(2):
                # rc = (MT5s @ G5s) + Fm  entirely in PSUM (PE-only cpr)
                rc = ps_tail.tile([128, 1024], F32, tag="rc",
                                  name=f"rc{q}{ct}")
                for hx in range(2):
                    h0 = q * 1024 + hx * 512
                    nc.tensor.matmul(rc[:, hx * 512:(hx + 1) * 512],
                                     MT5s[:, ct * 128:(ct + 1) * 128],
                                     G5s[:, h0:h0 + 512], start=True,
                                     stop=False)
                    nc.tensor.matmul(rc[:, hx * 512:(hx + 1) * 512],
                                     identb[:], Fm[:, ct, h0:h0 + 512],
                                     start=False, stop=True)
                rcb = sigp.tile([128, 1024], BF16, tag="rcb",
                                name=f"rcb{q}{ct}")
                nc.scalar.copy(rcb[:], rc[:])
                nc.gpsimd.dma_start(cpr_o[ct, :, q * 16:(q + 1) * 16, :],
                                    rcb[:].rearrange("p (h w) -> p h w",
                                                     h=16))
                A2 = tailp.tile([128, 16, 64], BF16, tag="A2",
                                name=f"A2{q}{ct}")
                nc.vector.tensor_tensor(
                    A2[:], PdS[:, ct, qs].rearrange("p (h w) -> p h w", h=16),
                    sigv, op=ALU.mult)
                fu = tailp.tile([128, 16, 64], BF16, tag="fu",
                                name=f"fu{q}{ct}")
                nc.vector.tensor_tensor(
                    fu[:], A2[:],
                    t2[:, ct, qs].rearrange("p (h w) -> p h w", h=16),
                    op=ALU.add)
                nc.sync.dma_start(fused_o[ct, :, q * 16:(q + 1) * 16, :],
                                  fu[:])

        pds_prep()
        tail_q(0)
        spat_q(2)
        sig_q(2)
        g5s_q(2)
        tail_q(1)
        spat_q(3)
        sig_q(3)
        g5s_q(3)
        tail_q(2)
        tail_q(3)
    ctx.close()


def _prep_weights(W3, b3, Wa1, ba1, Wa2, ba2, Wa3, ba3, Wu, bu, Wr, br,
                  Wsa, bsa, Wsm, bsm):
    f = np.float32
    # w3t[p, kt, t, co] = W3[co, kt*128+p, dy, dx]
    w3t = np.ascontiguousarray(
        W3.reshape(C, 4, 128, 9).transpose(2, 1, 3, 0)).astype(f)
    b3h = np.ascontiguousarray(b3.reshape(2, 128).T).astype(f)
    # adapter + U_gen collapsed: u = (Wu@Wa)@[avg;max] + (Wu@ba + bu)
    wkb = np.zeros((3, 3, 4), f)
    for mi, (Wa, ba, div) in enumerate(
            ((Wa1, ba1, HW), (Wa2, ba2, C * H), (Wa3, ba3, C * W))):
        wk = Wu @ Wa                               # [4, 2]
        wkb[mi, 0] = wk[:, 0] / float(div)
        wkb[mi, 1] = wk[:, 1]
        wkb[mi, 2] = Wu @ ba + bu
    wkb = np.ascontiguousarray(
        np.broadcast_to(wkb[None], (128, 3, 3, 4))).astype(f)
    # wrt[p, kk, mm, m] = Wr[mm*128+m, kk*128+p]
    wrt = np.ascontiguousarray(
        Wr.reshape(2, 128, 2, 128).transpose(3, 2, 0, 1)).astype(f)
    brh = br.reshape(1, 256).astype(f)
    # wsc[p, kk, mm, m]: kk<2 -> Wsa/128 (mean folded), kk>=2 -> Wsm
    wsa_r = (Wsa / 128.0).reshape(2, 128, 2, 128).transpose(3, 2, 0, 1)
    wsm_r = Wsm.reshape(2, 128, 2, 128).transpose(3, 2, 0, 1)
    wsc = np.ascontiguousarray(
        np.concatenate([wsa_r, wsm_r], axis=1)).astype(f)
    bsc = np.ascontiguousarray((bsa + bsm).reshape(2, 128).T).astype(f)
    import ml_dtypes
    bf = ml_dtypes.bfloat16
    return dict(w3t=w3t.astype(bf), bb=np.ascontiguousarray(
                    np.concatenate([b3h, bsc], axis=1)),
                wkb=wkb, wrt=wrt.astype(bf), br=brh.astype(bf),
                wsc=wsc.astype(bf), onesd=np.ones((1, HW), bf))


_CACHE = {}


def kernel(frm_feat, other_feat, W3, b3, Wa1, ba1, Wa2, ba2, Wa3, ba3,
           Wu, bu, Wr, br, ws, bs, Wsa, bsa, Wsm, bsm, alpha, lam,
           _trace=False, _tmpdir=None):
    import ml_dtypes
    bf = ml_dtypes.bfloat16
    frm_feat = np.asarray(frm_feat, np.float32)
    other_feat = np.asarray(other_feat, np.float32)
    key = (float(alpha), float(ws), float(bs))
    if key not in _CACHE:
        _CACHE[key] = build_program(float(alpha), float(ws), float(bs))
    nc = _CACHE[key]

    wd = _prep_weights(np.asarray(W3), np.asarray(b3), np.asarray(Wa1),
                       np.asarray(ba1), np.asarray(Wa2), np.asarray(ba2),
                       np.asarray(Wa3), np.asarray(ba3), np.asarray(Wu),
                       np.asarray(bu), np.asarray(Wr), np.asarray(br),
                       np.asarray(Wsa), np.asarray(bsa), np.asarray(Wsm),
                       np.asarray(bsm))
    wd["lam"] = np.asarray(lam, np.float32).reshape(4, 1)

    in_maps = []
    for b_i in range(NCORES):
        m = dict(wd)
        xin = np.zeros((128, 4, 66, 66), bf)
        xin[:, 0:2, 1:65, 1:65] = frm_feat[b_i].reshape(
            2, 128, 64, 64).transpose(1, 0, 2, 3).astype(bf)
        xin[:, 2:4, 1:65, 1:65] = other_feat[b_i].reshape(
            2, 128, 64, 64).transpose(1, 0, 2, 3).astype(bf)
        m["xin"] = xin
        in_maps.append(m)

    res = bass_utils.run_bass_kernel_spmd(
        nc, in_maps, core_ids=list(range(NCORES)), trace=_trace,
        tmpdir=_tmpdir)
    fused = np.stack([
        np.asarray(res.results[i]["fused"]).astype(np.float32).reshape(C, H, W)
        for i in range(NCORES)])
    cpr = np.stack([
        np.asarray(res.results[i]["cpr"]).astype(np.float32).reshape(C, H, W)
        for i in range(NCORES)])
    kernel._last_exec_time_ns = res.exec_time_ns
    kernel._last_results = res
    return fused, cpr
